# revision 32
# baseline (speedup 1.0000x reference)
"""Bottom-Up Hidden Tree Markov Model upward pass on 8 Trainium2 NeuronCores.

Problem: complete 8-ary forest (2 trees x 299593 nodes, depth 6), C=8 hidden
states, 32 symbols, 16 independent generative models. Output: per-tree
log-likelihood (2, 16).

Sharding: core = (tree, quarter-of-tree). Each core runs the upward pass over
its quarter for the two big levels: 65536 leaves -> 8192 depth-5 betas (via
one-hot(symbol) matmuls against a folded leaf table T6) -> 1024 depth-4
betas. The host finishes depths 3..0 (146 nodes/core, f64) plus the leaf
log-scale term (histogram x log-table, exact in f64).

Device structure per level: tb via matmuls; bl = tb * bx elementwise
(bx = B[:, x_parent] gathered host-side, resident in SBUF); per-node
normalizer nu via a 16-row selector matmul, 4 chunks packed per PSUM bank at
32-partition offsets so one Ln (free-dim accumulation -> ll partial) and one
reciprocal serve 4 chunks; normalized beta = bl * broadcast(1/nu) with the
broadcast done by a positioned 16-row identity matmul.
Partition packing everywhere: p = i*16 + g  (i = hidden state, g = generator).
"""
import sys

import numpy as np

if '/opt/trn_rl_repo' not in sys.path:
    sys.path.insert(0, '/opt/trn_rl_repo')

import ml_dtypes

BF16 = ml_dtypes.bfloat16

K, DEPTH, NTREE, C, MSYM, NGEN = 8, 6, 2, 8, 32, 16
STARTS = [(K ** d - 1) // (K - 1) for d in range(DEPTH + 2)]
NT = STARTS[DEPTH + 1]          # 299593 nodes per tree
CG = C * NGEN                   # 128
NQ = 4                          # quarters per tree
LEAVES_Q = (K ** DEPTH) // NQ   # 65536 leaves per core
NP5 = LEAVES_Q // K             # 8192 depth-5 parents per core
NP4 = NP5 // K                  # 1024 depth-4 parents per core
NBX = NP5 + NP4                 # interior symbol-gather columns per core
NCHUNK = 512                    # parents per chunk (one PSUM bank of f32)
POOL_OUT = frozenset({1, 3, 5, 7, 9, 11, 13, 15})  # L0 chunks normalized on Pool


def _softmax64(x, axis):
    x = np.asarray(x, np.float64)
    e = np.exp(x - x.max(axis=axis, keepdims=True))
    return e / e.sum(axis=axis, keepdims=True)


def _build_tables(A, B, Pi, SP):
    """Small O(params) tables, f64 on host."""
    smA = _softmax64(A, 0)            # (C,C,K,G) over parent state i
    smB = _softmax64(B, 1)            # (C,M,G) over symbols
    smPi = _softmax64(Pi, 0)          # (C,K,G)
    smSP = _softmax64(SP, 0)          # (K,G)
    Mmat = smSP[:, None, None, :] * np.transpose(smA, (2, 0, 1, 3))  # [l,i,j,g]
    pb = smPi[:, :, None, :] * smB[:, None, :, :]     # (j, l, s, g)
    nuL = pb.sum(0)                                    # (l, s, g)
    betaLeaf = pb / nuL[None]
    llLeaf = np.log(nuL)                               # (l, s, g)
    T6 = np.einsum('lijg,jlsg->lsig', Mmat, betaLeaf)  # (l,s,i,g)
    T6f = T6.reshape(K * MSYM, CG)                     # rows (l*32+s), cols (i,g)
    # reorder rows for the 16-broadcast one-hot layout: partition p = l*16+sh
    p = np.arange(CG)
    idxA = (p // 16) * MSYM + (p % 16)
    idxB = (p // 16) * MSYM + 16 + (p % 16)
    Wl = np.zeros((K, CG, CG))
    ii = np.arange(C)
    for l in range(K):
        for g in range(NGEN):
            Wl[l, ii[:, None] * NGEN + g, ii[None, :] * NGEN + g] = Mmat[l, :, :, g].T
    BT = np.transpose(smB, (1, 0, 2)).reshape(MSYM, CG)  # row s -> B[i,s,g] at i*16+g
    E16 = (p[None, :] % NGEN == np.arange(NGEN)[:, None]).astype(np.float64)  # [16,128]
    E16x4 = np.concatenate(
        [np.vstack([E16, np.zeros((16, CG))]) for _ in range(4)], axis=0)  # [128,128]
    selt = (p[:, None] % NGEN == np.arange(NGEN)[None, :]).astype(np.float64)  # [128,16]
    Wt = np.concatenate([Wl[l] for l in range(K)], axis=1)  # [128, 1024]
    packed = np.concatenate([T6f[idxA], T6f[idxB], E16x4, selt, Wt], axis=1)
    tabs = {
        'tabs': np.ascontiguousarray(packed.astype(BF16)),   # [128, 1424]
        'svh': np.stack([p % 16, p % 16 + 16], axis=1).astype(np.float32),  # [128, 2]
    }
    host = {'Mmat': Mmat, 'smB': smB, 'BT': BT, 'llLeaf': llLeaf}
    return tabs, host


def _build_bass(n_reps=1):
    import concourse.bass as bass
    import concourse.bacc as bacc
    import concourse.mybir as mybir
    from concourse import tile

    f32 = mybir.dt.float32
    bf16 = mybir.dt.bfloat16
    Alu = mybir.AluOpType
    Act = mybir.ActivationFunctionType

    nc = bacc.Bacc(None, target_bir_lowering=False)

    # packed bf16 tables: [T6a | T6b | E16x4 | selt | Wt] along columns
    TABW = CG + CG + CG + 16 + 1024   # 1424
    OFF_T6A, OFF_T6B, OFF_E, OFF_SEL, OFF_W = 0, CG, 2 * CG, 3 * CG, 3 * CG + 16
    xs_d = nc.dram_tensor('xs16', [K, NP5], bf16, kind='ExternalInput')
    bxh_d = nc.dram_tensor('bxh', [CG, NBX], bf16, kind='ExternalInput')
    tabs_d = nc.dram_tensor('tabs', [CG, TABW], bf16, kind='ExternalInput')
    svh_d = nc.dram_tensor('svh', [CG, 2], f32, kind='ExternalInput')
    bl4_d = nc.dram_tensor('bl4', [CG, NP4], bf16, kind='ExternalOutput')
    llp_d = nc.dram_tensor('llp', [CG, 4], f32, kind='ExternalOutput')

    GRP = 2048          # parents per one-hot group (4 chunks = 1 nu bank)
    NGRP = NP5 // GRP   # 4

    with tile.TileContext(nc) as tc:
      for _rep in range(n_reps):
        with (
            tc.tile_pool(name='const', bufs=1) as constp,
            tc.tile_pool(name='big', bufs=1) as bigp,
            tc.tile_pool(name='xsb', bufs=2) as xsbp,
            tc.tile_pool(name='oh', bufs=4) as ohp,
            tc.tile_pool(name='tbsb', bufs=2) as tbsbp,
            tc.tile_pool(name='blg', bufs=3) as blgp,
            tc.tile_pool(name='bl1', bufs=2) as bl1p,
            tc.tile_pool(name='rr', bufs=2) as rrp,
            tc.tile_pool(name='lns', bufs=2) as lnsp,
            tc.tile_pool(name='ps_tb', bufs=2, space='PSUM') as ps_tb,
            tc.tile_pool(name='ps_nu', bufs=2, space='PSUM') as ps_nu,
            tc.tile_pool(name='ps_rb', bufs=2, space='PSUM') as ps_rb,
        ):
            tabs = constp.tile([CG, TABW], bf16, tag='tabs', name='tabs')
            svh = constp.tile([CG, 2], f32, tag='svh', name='svh')
            bxh = bigp.tile([CG, NBX], bf16, tag='bxh', name='bxh')
            b5 = bigp.tile([CG, NP5], bf16, tag='b5', name='b5')
            bl4sb = bigp.tile([CG, NP4], bf16, tag='bl4sb', name='bl4sb')
            llp = bigp.tile([CG, 4], f32, tag='llp', name='llp')

            # interleave input streaming; first xs piece is small so group 0
            # can start early, the rest stream in bigger pieces
            xs_pieces = [(0, GRP), (GRP, GRP), (2 * GRP, 2 * GRP)]
            xsb_t = []
            for pi, (xo, xw) in enumerate(xs_pieces):
                xsb = xsbp.tile([CG, xw], bf16, tag=f'xsb{pi}', name=f'xsb{pi}')
                src = bass.AP(xs_d[:].tensor, xo, [[NP5, K], [0, 16], [1, xw]])
                nc.sync.dma_start(xsb[:], src)
                xsb_t.append(xsb)
                if pi == 0:
                    nc.sync.dma_start(svh[:], svh_d[:])
                    nc.sync.dma_start(tabs[:], tabs_d[:])
                    nc.sync.dma_start(bxh[:, 0:GRP], bxh_d[:, 0:GRP])
                    svh1 = svh[:, 0:1]
                    svh2 = svh[:, 1:2]
                elif pi == 1:
                    nc.sync.dma_start(bxh[:, GRP:2 * GRP], bxh_d[:, GRP:2 * GRP])
            nc.sync.dma_start(bxh[:, 2 * GRP:3 * GRP], bxh_d[:, 2 * GRP:3 * GRP])
            nc.sync.dma_start(bxh[:, 3 * GRP:NBX], bxh_d[:, 3 * GRP:NBX])

            # preload the activation table set containing Ln (and Copy) so the
            # mid-stream LoadActFuncSet does not land on the critical path
            warm = lnsp.tile([CG, 1], f32, tag='warm', name='warm')
            nc.scalar.activation(warm[:], svh2, Act.Ln)

            # 2 alternating nu banks; filler rows (16..31 of each 32-block)
            # are set to 1.0 once up-front (ln -> 0); matmul start=True only
            # resets the 16-row regions it writes, so the filler persists
            # across bank reuse.  These memsets and the one-hot builds below
            # run while the input DMAs stream in.
            nu_banks = [ps_nu.tile([CG, NCHUNK], f32, tag='nu', name=f'nu{b}')
                        for b in range(2)]
            for b in range(2):
                nc.vector.memset(nu_banks[b][:], 1.0)

            ohs = []
            for pi, (xo, xw) in enumerate(xs_pieces):
                oh1 = ohp.tile([CG, xw], bf16, tag=f'oh{pi}', name=f'oh1_{pi}')
                oh2 = ohp.tile([CG, xw], bf16, tag=f'oh{pi}', name=f'oh2_{pi}')
                nc.vector.tensor_scalar(oh1[:], xsb_t[pi][:], svh1, None,
                                        Alu.is_equal)
                nc.vector.tensor_scalar(oh2[:], xsb_t[pi][:], svh2, None,
                                        Alu.is_equal)
                ohs.append((oh1, oh2))

            b5v = b5[:].rearrange('p (u l) -> p u l', l=K)

            NHC = NP4 // 2   # 512 level-1 parents per half

            NQC = NP4 // 4   # 256 level-1 parents per quarter

            def level1_quarter(c):
                tb_ps = ps_rb.tile([CG, NQC], f32, tag='rb', name='tb1q')
                for l in range(K):
                    nc.tensor.matmul(tb_ps[:], tabs[:, OFF_W + CG * l:OFF_W + CG * (l + 1)],
                                     b5v[:, c * NQC:(c + 1) * NQC, l],
                                     start=(l == 0), stop=(l == K - 1))
                nc.vector.tensor_mul(bl4sb[:, c * NQC:(c + 1) * NQC], tb_ps[:],
                                     bxh[:, NP5 + c * NQC:NP5 + (c + 1) * NQC])
                nc.sync.dma_start(bl4_d[:, c * NQC:(c + 1) * NQC],
                                  bl4sb[:, c * NQC:(c + 1) * NQC])

            def level1_half(c):
                # children of depth-4 parents 512c..512c+511 are b5 groups
                # 2c, 2c+1, so this can run right after phaseB(2c+1)
                tb_ps = ps_rb.tile([CG, NHC], f32, tag='rb', name='tb1')
                for l in range(K):
                    nc.tensor.matmul(tb_ps[:], tabs[:, OFF_W + CG * l:OFF_W + CG * (l + 1)],
                                     b5v[:, c * NHC:(c + 1) * NHC, l],
                                     start=(l == 0), stop=(l == K - 1))
                nc.vector.tensor_mul(bl4sb[:, c * NHC:(c + 1) * NHC], tb_ps[:],
                                     bxh[:, NP5 + c * NHC:NP5 + (c + 1) * NHC])
                nc.sync.dma_start(bl4_d[:, c * NHC:(c + 1) * NHC],
                                  bl4sb[:, c * NHC:(c + 1) * NHC])

            # ---- level 0: 8192 depth-5 parents; 4 groups of 4 chunks ----
            # software-pipelined: compute phase of group g overlaps the
            # normalize phase of group g-1
            blgs = {}

            r_sbs = {}

            def a_pair(g4, pp):
                # chunk pair pp: T6 matmuls into a 2-bank psum tile, one
                # 1024-wide copy to SBUF, one 1024-wide bl multiply
                oh1, oh2 = ohs[g4] if g4 < 2 else ohs[2]
                go = 0 if g4 < 2 else (g4 - 2) * GRP
                po = pp * 2 * NCHUNK
                tb_ps = ps_tb.tile([CG, 2 * NCHUNK], f32, tag='tb', name='tb')
                for s in range(2):
                    co = po + s * NCHUNK
                    nc.tensor.matmul(tb_ps[:, s * NCHUNK:(s + 1) * NCHUNK],
                                     tabs[:, OFF_T6A:OFF_T6A + CG],
                                     oh1[:, go + co:go + co + NCHUNK],
                                     start=True, stop=False)
                    nc.tensor.matmul(tb_ps[:, s * NCHUNK:(s + 1) * NCHUNK],
                                     tabs[:, OFF_T6B:OFF_T6B + CG],
                                     oh2[:, go + co:go + co + NCHUNK],
                                     start=False, stop=True)
                tbsb, blg = blgs[g4]
                nc.scalar.copy(tbsb[:, po:po + 2 * NCHUNK], tb_ps[:])
                eng = nc.vector if pp == 1 else nc.gpsimd
                eng.tensor_mul(blg[:, po:po + 2 * NCHUNK], tbsb[:, po:po + 2 * NCHUNK],
                               bxh[:, g4 * GRP + po:g4 * GRP + po + 2 * NCHUNK])

            def emit_nu(g4, cc):
                _, blg = blgs[g4]
                co = cc * NCHUNK
                nc.tensor.matmul(nu_banks[g4 % 2][32 * cc:32 * cc + 16, :],
                                 tabs[:, OFF_SEL:OFF_SEL + 16],
                                 blg[:, co:co + NCHUNK],
                                 start=True, stop=True, tile_position=(0, 32 * cc))

            def emit_recip(g4):
                r_sb = rrp.tile([CG, NCHUNK], bf16, tag='r', name='r')
                with nc.allow_low_precision(reason='bf16 normalizer broadcast; validated vs reference'):
                    nc.vector.reciprocal(r_sb[:], nu_banks[g4 % 2][:])
                r_sbs[g4] = r_sb

            def b_chunk(g4, cc):
                c = g4 * 4 + cc
                poff = 32 * cc
                r_sb = r_sbs[g4]
                _, blg = blgs[g4]
                rb_ps = ps_rb.tile([CG, NCHUNK], f32, tag='rb', name='rb')
                nc.tensor.matmul(rb_ps[:], tabs[poff:poff + 16, OFF_E:OFF_E + CG],
                                 r_sb[poff:poff + 16, :],
                                 start=True, stop=True, tile_position=(poff, 0))
                nc.vector.tensor_mul(b5[:, c * NCHUNK:(c + 1) * NCHUNK],
                                     blg[:, cc * NCHUNK:(cc + 1) * NCHUNK], rb_ps[:])

            for g4 in range(NGRP):
                blgs[g4] = (tbsbp.tile([CG, GRP], bf16, tag='tbsb', name='tbsb'),
                            blgp.tile([CG, GRP], bf16, tag='blg', name='blg'))
                if g4 > 0:
                    emit_recip(g4 - 1)
                a_pair(g4, 0)
                emit_nu(g4, 0)
                emit_nu(g4, 1)
                if g4 > 0:
                    b_chunk(g4 - 1, 0)
                    b_chunk(g4 - 1, 1)
                a_pair(g4, 1)
                emit_nu(g4, 2)
                emit_nu(g4, 3)
                if g4 > 0:
                    b_chunk(g4 - 1, 2)
                    b_chunk(g4 - 1, 3)
                lns = lnsp.tile([CG, NCHUNK], f32, tag='lns', name='lns')
                nc.scalar.activation(lns[:], nu_banks[g4 % 2][:], Act.Ln,
                                     accum_out=llp[:, g4:g4 + 1])
                if g4 == 2:
                    level1_half(0)
            emit_recip(3)
            for cc in range(4):
                b_chunk(3, cc)
            level1_half(1)
            nc.sync.dma_start(llp_d[:], llp[:])
    if not nc.is_finalized():
        nc.finalize()
    return nc


_BASS_CACHE = {}


def _get_bass():
    if 'nc' not in _BASS_CACHE:
        _BASS_CACHE['nc'] = _build_bass()
    return _BASS_CACHE['nc']


def kernel(**inputs):
    from concourse.bass_utils import run_bass_kernel_spmd

    A = np.asarray(inputs['A']); B = np.asarray(inputs['B'])
    Pi = np.asarray(inputs['Pi']); SP = np.asarray(inputs['SP'])
    x = np.asarray(inputs['x'])

    tabs, host = _build_tables(A, B, Pi, SP)
    Mmat, smB, BT, llLeaf = host['Mmat'], host['smB'], host['BT'], host['llLeaf']

    in_maps = []
    for t in range(NTREE):
        base = t * NT
        for q in range(NQ):
            s6 = base + STARTS[6] + q * LEAVES_Q
            xs = x[s6: s6 + LEAVES_Q]
            xs16 = np.ascontiguousarray(
                xs.reshape(NP5, K).T.astype(BF16))          # [8, 8192]
            s5 = base + STARTS[5] + q * NP5
            s4 = base + STARTS[4] + q * NP4
            xi = np.concatenate([x[s5: s5 + NP5], x[s4: s4 + NP4]])
            bxh = np.ascontiguousarray(BT[xi].T.astype(BF16))  # [128, 9216]
            m = {'xs16': xs16, 'bxh': bxh}
            m.update(tabs)
            in_maps.append(m)

    nc = _get_bass()
    global _LAST_IN_MAPS
    _LAST_IN_MAPS = in_maps
    res = run_bass_kernel_spmd(nc, in_maps, core_ids=list(range(8)))
    results = res.results

    out = np.zeros((NTREE, NGEN), np.float64)
    for t in range(NTREE):
        base = t * NT
        # device ll partials (level 6->5); partition p = 32*block + g for
        # g < 16, filler rows hold ln(1.0) = 0
        for q in range(NQ):
            llp = np.asarray(results[t * NQ + q]['llp'], np.float64)  # [128, 4]
            out[t] += llp.reshape(4, 32, 4).sum(axis=(0, 2))[:16]
        # leaf log-scale term: histogram x log-table (exact)
        xleaf = x[base + STARTS[6]: base + STARTS[6] + K ** DEPTH]
        pos = np.tile(np.arange(K), K ** (DEPTH - 1))
        cnt = np.bincount(pos * MSYM + xleaf, minlength=K * MSYM).astype(np.float64)
        out[t] += cnt @ llLeaf.reshape(K * MSYM, NGEN)
        # depth-4: normalize the device's unnormalized bl4 here (f64, exact)
        beta = np.empty((NQ, NP4, C, NGEN), np.float64)
        for q in range(NQ):
            bl4 = np.asarray(results[t * NQ + q]['bl4'], np.float64)  # [128, 1024]
            bl4 = bl4.T.reshape(NP4, C, NGEN)
            nu4 = bl4.sum(1)
            out[t] += np.log(nu4).sum(axis=0)
            beta[q] = bl4 / nu4[:, None]
        bcur = beta.reshape(NQ * NP4, C, NGEN)  # tree t depth-4 level, in order
        for d in (3, 2, 1):
            nd = K ** d
            bch = bcur.reshape(nd, K, C, NGEN)
            tb = np.einsum('uljg,lijg->uig', bch, Mmat)
            sd = base + STARTS[d]
            bl = tb * np.transpose(smB[:, x[sd: sd + nd]], (1, 0, 2))
            nu = bl.sum(1)
            out[t] += np.log(nu).sum(axis=0)
            bcur = bl / nu[:, None]
        # root: combine the 8 depth-1 betas
        tb0 = np.einsum('ljg,lijg->ig', bcur, Mmat)
        bl0 = tb0 * smB[:, x[base]]
        nu0 = bl0.sum(0)
        out[t] += np.log(nu0)
    return out.astype(np.float32)
